# revision 4
# baseline (speedup 1.0000x reference)
"""Grouped-experts SwiGLU MoE kernel for Trainium2 (8 NeuronCores).

Problem: T=8192 tokens (pre-sorted into contiguous per-expert blocks of
sizes num_tokens_per_expert), D=1024, H=2816, E=8 experts.
out[t] = (silu(x@w1^T) * (x@w3^T)) @ w2^T  with the owning expert's weights;
tokens past sum(counts) produce zeros.

Sharding: 8-way tensor-parallel split of the hidden dim H, exact 352
per core.  Every core processes ALL valid tokens of ALL experts for its
h-slice and emits partial outputs (contraction over h is split); the
host sums the 8 partials.  Every core's instruction stream is identical
(true SPMD) -- perfectly load-balanced regardless of expert imbalance.

GEMMs run in bf16 (PE 1 cycle/row) with fp32 PSUM accumulation.

All GEMMs stream packed token columns (moving dim = tokens, <=512 per
chunk).  GEMM1/3 use the 3-h-slot form: slots cover h ranges
(0,128),(128,256),(224,352); the third slot overlaps by 32 rows, which
are zero-weighted in every expert's w2 k2 tile, so all 128 partitions
of every h2 plane carry defined values at zero extra PE cost.  GEMM2
runs in [d, token] form: g d-tiles share one PSUM bank so one DVE copy
drains them all.

No DMA transposes anywhere: HW-measured transpose cost is ~1.2-3.6us
each (vs ~0.2us modeled), so the transpose-based stationary-x
formulation loses ~100us/rep of HWDGE ring time to save ~7us of PE.

Scheduling:
- Software pipelining: GEMM2(prev expert) is emitted after GEMM1/3(curr
  expert), so the PE never waits on the silu/mul chain; w2(e) loads are
  deferred by one expert as well.
- GEMM2's PSUM drain alternates between DVE and Act (activation-Copy):
  with DVE alone the drain rate (~0.6us per 512-col bank, plus the
  interleaved h2 muls) trails the PE's ~0.6us refill rate, so the PE
  stalls on PSUM-bank reuse -- HW-measured at ~48us/rep (217.7 ->
  169.5us med-slope at a 96-rep span).
- out DRAM layout puts the d-block dim innermost -> every store DMA is
  128 descriptors of large contiguous per-partition segments.
- DMA spread: x + w2 on SP (HWDGE), w1/w3 on Pool (SWDGE), big-expert
  stores on Pool, small on Act, the final store on SP (lower completion
  latency in the end-of-kernel drain).
"""

import sys

sys.path.insert(0, "/opt/trn_rl_repo")

import numpy as np
import ml_dtypes

T, D, E = 8192, 1024, 8
H = 2816
CAP = T // E
NCORES = 8
HT = 3  # h-slots per core, ranges (0,128),(128,256),(224,352)
HSLICE = H // NCORES  # 352 exact, no padding
BF16 = ml_dtypes.bfloat16

_COMPILE_CACHE = {}
LAST_RESULTS = None  # BassKernelResults of the most recent device run


def _derive_cfg(counts):
    """Static structure derived from the per-expert token counts.
    Tokens are packed exactly (no padding): expert e owns packed columns
    [offs[e], offs[e]+counts[e]).  All GEMMs consume near-equal chunks of
    <=512 columns."""
    counts = [int(c) for c in counts]
    offs = [0]
    for c in counts:
        offs.append(offs[-1] + c)
    total_cols = offs[-1]
    order = sorted([e for e in range(E) if counts[e] > 0],
                   key=lambda e: -counts[e])

    def _mkchunks(e, maxw):
        c = counts[e]
        out = []
        n = -(-c // maxw)
        base, rem = divmod(c, n)
        c0 = 0
        for i in range(n):
            w = base + (1 if i < rem else 0)
            out.append((offs[e] + c0, w))
            c0 += w
        return out

    chunks = {e: (_mkchunks(e, 512) if counts[e] else []) for e in range(E)}
    return {
        "counts": counts,
        "offs": offs[:E],
        "total_cols": total_cols,
        "chunks": chunks,
        "order": order,
    }


def _build_program(cfg, repeat=1, hw_loop=False):
    import concourse.bass as bass
    import concourse.bacc as bacc
    import concourse.mybir as mybir
    import concourse.tile as tile

    dt = mybir.dt
    COLS = cfg["total_cols"]
    counts = cfg["counts"]
    offs = cfg["offs"]
    chunks = cfg["chunks"]
    order = cfg["order"]

    nc = bacc.Bacc("TRN2", target_bir_lowering=False, debug=False,
                   num_devices=NCORES)

    # x packed columns: [p=128, do=8, col] with d = do*128 + p; cols
    # innermost so the slot-form MOVING operand xe[:, d, c0:c0+w] is a
    # contiguous 2B-stride stream (strided moving reads run ~4x slower
    # on HW); each x DMA is 128x8 descriptors of 2w-byte segments
    xts = nc.dram_tensor("xts", [128, 8, COLS], dt.bfloat16,
                         kind="ExternalInput").ap()
    # weights pre-permuted on host; every DMA slice is contiguous per
    # partition row:
    # w1s/w3s: (E, p=128, do=8, h=352)  [d = do*128+p contracted; h exact]
    # w2s:     (E, p=128, kt=3, d=D)    [h = kt*128+p contracted;
    #                                    kt=2 rows 0:32 are zero pad]
    w1s = nc.dram_tensor("w1s", [E, 128, 8, HSLICE], dt.bfloat16,
                         kind="ExternalInput").ap()
    w3s = nc.dram_tensor("w3s", [E, 128, 8, HSLICE], dt.bfloat16,
                         kind="ExternalInput").ap()
    w2s = nc.dram_tensor("w2s", [E, 128, HT, D], dt.bfloat16,
                         kind="ExternalInput").ap()
    # partial output, [p=128, col, dt=8]: out[col, dt*128+p]
    outp = nc.dram_tensor("outp", [128, COLS, 8], dt.bfloat16,
                          kind="ExternalOutput").ap()

    with tile.TileContext(nc) as tc:
        with (
            tc.tile_pool(name="xpool", bufs=2) as xpool,
            tc.tile_pool(name="w1pool", bufs=2) as w1pool,
            tc.tile_pool(name="w3pool", bufs=2) as w3pool,
            tc.tile_pool(name="w2pool", bufs=3) as w2pool,
            tc.tile_pool(name="h2pool", bufs=3) as h2pool,
            tc.tile_pool(name="sgpool", bufs=3) as sgpool,
            tc.tile_pool(name="obpool", bufs=2) as obpool,
            tc.tile_pool(name="psgu", bufs=2, space="PSUM") as psgu,
            tc.tile_pool(name="pso", bufs=4, space="PSUM") as pso,
        ):
          def _body():
            state = {}  # e -> (xe, w1t, w3t, w2t, h2)

            def emit_loads(e, pending_w2):
                xe = xpool.tile([128, 8, 1024], dt.bfloat16, tag="xe")
                w1t = w1pool.tile([128, 8, HSLICE], dt.bfloat16, tag="w1t")
                w3t = w3pool.tile([128, 8, HSLICE], dt.bfloat16, tag="w3t")
                w2t = w2pool.tile([128, HT, D], dt.bfloat16, tag="w2t")
                # x chunks + w2 on the SP queue; w1/w3 on the Pool (SWDGE)
                # queue: the queues transfer in parallel.
                nc.gpsimd.dma_start(w1t[:, :, :], w1s[e][:, :, :])
                for (col0, w) in chunks[e]:
                    rel0 = col0 - offs[e]
                    nc.sync.dma_start(xe[:, :, rel0:rel0 + w],
                                      xts[:, :, col0:col0 + w])
                nc.gpsimd.dma_start(w3t[:, :, :], w3s[e][:, :, :])
                # w2 of the PREVIOUS expert after this expert's x (not
                # needed until its GEMM2, which runs an expert later)
                for (pe, pw2) in pending_w2:
                    nc.sync.dma_start(pw2[:, :, :], w2s[pe][:, :, :])
                state[e] = (xe, w1t, w3t, w2t, None)
                return w2t

            def emit_g13(e):
                xe, w1t, w3t, w2t, _ = state[e]
                h2 = h2pool.tile([128, HT, 1024], dt.bfloat16, tag="h2")
                # 3-h-slot form: slots (0,128),(128,256),(224,352); slot 3
                # overlaps by 32 rows, zero-weighted in w2's k2 tile
                for h, hc0 in enumerate((0, 128, 224)):
                    for (col0, w) in chunks[e]:
                        rel0 = col0 - offs[e]
                        pg = psgu.tile([128, 512], dt.float32, tag="pg")
                        pu = psgu.tile([128, 512], dt.float32, tag="pu")
                        for d in range(8):
                            nc.tensor.matmul(
                                pg[:, :w], w1t[:, d, hc0:hc0 + 128],
                                xe[:, d, rel0:rel0 + w],
                                start=(d == 0), stop=(d == 7))
                        for d in range(8):
                            nc.tensor.matmul(
                                pu[:, :w], w3t[:, d, hc0:hc0 + 128],
                                xe[:, d, rel0:rel0 + w],
                                start=(d == 0), stop=(d == 7))
                        sg = sgpool.tile([128, 512], dt.float32, tag="sg")
                        nc.scalar.activation(
                            sg[:, :w], pg[:, :w],
                            mybir.ActivationFunctionType.Silu)
                        nc.vector.tensor_mul(
                            out=h2[:, h, rel0:rel0 + w],
                            in0=sg[:, :w], in1=pu[:, :w])
                state[e] = (xe, w1t, w3t, w2t, h2)

            def emit_g2(e, last=False):
                _, _, _, w2t, h2 = state[e]
                for (col0, w) in chunks[e]:
                    rel0 = col0 - offs[e]
                    # pack g d-tiles into one PSUM bank -> one copy per
                    # group (amortizes the per-copy fixed cost); the very
                    # last chunk caps g at 4 so its final copy overlaps PE
                    g = 8 if w <= 64 else 4 if w <= 128 else 2 if w <= 256 else 1
                    if last and (col0, w) == chunks[e][-1]:
                        g = min(g, 4)
                    ob = obpool.tile([128, 512, 8], dt.bfloat16, tag="ob")
                    for dt0 in range(0, 8, g):
                        po = pso.tile([128, 512], dt.float32, tag="po")
                        for gi in range(g):
                            dti = dt0 + gi
                            for k in range(HT):
                                nc.tensor.matmul(
                                    po[:, gi * w:gi * w + w],
                                    w2t[:, k, dti * 128:(dti + 1) * 128],
                                    h2[:, k, rel0:rel0 + w],
                                    start=(k == 0), stop=(k == HT - 1))
                        # ob view (dt, col): permute so it matches po's
                        # flat (dt-major) layout; alternate the PSUM drain
                        # between DVE and Act so neither engine's queue
                        # backlog ever stalls the PE via pso reuse
                        obv = ob[:, :w, dt0:dt0 + g].transpose([0, 2, 1])
                        if (dt0 // g) % 2 == 0:
                            nc.vector.tensor_copy(out=obv, in_=po[:, :g * w])
                        else:
                            nc.scalar.activation(
                                obv, po[:, :g * w],
                                mybir.ActivationFunctionType.Copy)
                    if last:
                        eng = nc.sync
                    elif counts[e] < 320:
                        eng = nc.scalar
                    else:
                        eng = nc.gpsimd
                    eng.dma_start(outp[:, col0:col0 + w, :],
                                  ob[:, :w, :])

            pending_w2 = []
            for i, e in enumerate(order):
                w2t = emit_loads(e, pending_w2)
                pending_w2 = [(e, w2t)]
                emit_g13(e)
                if i >= 1:
                    emit_g2(order[i - 1])
            if order:
                # last expert's w2 load was deferred; issue it now
                for (pe, pw2) in pending_w2:
                    nc.sync.dma_start(pw2[:, :, :], w2s[pe][:, :, :])
                emit_g2(order[-1], last=True)

          if hw_loop and repeat > 1:
            with tc.For_i(0, repeat, 1):
                _body()
          else:
            for _rep in range(repeat):
                _body()

    nc.compile()
    return nc


def _get_program(cfg, repeat=1, hw_loop=False):
    key = (tuple(cfg["counts"]), repeat, hw_loop)
    if key not in _COMPILE_CACHE:
        _COMPILE_CACHE[key] = _build_program(cfg, repeat, hw_loop)
    return _COMPILE_CACHE[key]


def _pack_inputs(x, counts, w1, w2, w3, cfg):
    """Build per-core input maps (host-side routing + layout)."""
    offs, COLS = cfg["offs"], cfg["total_cols"]

    # packed x: all valid tokens, exactly packed per expert
    xpack = np.zeros((COLS, D), np.float32)
    starts = np.concatenate([[0], np.cumsum(counts)]).astype(np.int64)
    for e in range(E):
        c = int(counts[e])
        if c:
            xpack[offs[e]:offs[e] + c] = x[starts[e]:starts[e] + c]
    # (COLS, D) -> (col, do=8, p=128) -> (p, do, col)
    xts = np.ascontiguousarray(
        xpack.astype(BF16).reshape(COLS, 8, 128).transpose(2, 1, 0))

    # weights: transpose so the contraction dim leads; exact H split
    # (HSLICE=352/core); w2 k-tiles padded to 128 rows with zeros
    w1b = w1.astype(BF16)
    w3b = w3.astype(BF16)
    w2b = w2.astype(BF16)
    # (E, D, H)
    w1T = np.ascontiguousarray(np.transpose(w1b, (0, 2, 1)))
    w3T = np.ascontiguousarray(np.transpose(w3b, (0, 2, 1)))
    # (E, H, D)
    w2T = np.ascontiguousarray(np.transpose(w2b, (0, 2, 1)))

    in_maps = []
    for c in range(NCORES):
        hs = slice(c * HSLICE, (c + 1) * HSLICE)
        # w1/w3: (D, HSLICE) -> (do, p, h) -> (p, do, h)
        w1c = w1T[:, :, hs].reshape(E, 8, 128, HSLICE).transpose(0, 2, 1, 3)
        w3c = w3T[:, :, hs].reshape(E, 8, 128, HSLICE).transpose(0, 2, 1, 3)
        # w2: (HSLICE, D) -> 3 k-tiles of 128 rows -> (p, kt, d).
        # k2 = h[224:352] for every expert, with the first 32 rows zeroed
        # (that h range is counted in k1; the overlap matches the h2
        # k2-plane produced by the slot form)
        w2p = np.zeros((E, HT, 128, D), BF16)
        w2p[:, 0] = w2T[:, hs, :][:, 0:128]
        w2p[:, 1] = w2T[:, hs, :][:, 128:256]
        w2p[:, 2, 32:128] = w2T[:, hs, :][:, 256:352]
        w2c = w2p.transpose(0, 2, 1, 3)
        in_maps.append({
            "xts": xts,
            "w1s": np.ascontiguousarray(w1c),
            "w3s": np.ascontiguousarray(w3c),
            "w2s": np.ascontiguousarray(w2c),
        })
    return in_maps, starts


def _unpack_output(results, counts, cfg, starts):
    offs = cfg["offs"]
    COLS = cfg["total_cols"]
    acc = np.zeros((COLS, D), np.float32)
    for r in results:
        # outp: (p, col, dt) with d = dt*128 + p -> (col, d)
        acc += r["outp"].astype(np.float32).transpose(1, 2, 0).reshape(COLS, D)
    out = np.zeros((T, D), np.float32)
    for e in range(E):
        c = int(counts[e])
        if c:
            out[starts[e]:starts[e] + c] = acc[offs[e]:offs[e] + c]
    return out


def kernel(x, num_tokens_per_expert, w1, w2, w3):
    global LAST_RESULTS
    counts = np.asarray(num_tokens_per_expert).astype(np.int64)
    cfg = _derive_cfg(counts)
    if cfg["total_cols"] == 0:
        return np.zeros((T, D), np.float32)

    nc = _get_program(cfg)
    in_maps, starts = _pack_inputs(
        np.asarray(x, np.float32), counts,
        np.asarray(w1, np.float32), np.asarray(w2, np.float32),
        np.asarray(w3, np.float32), cfg)

    from concourse.bass_utils import run_bass_kernel_spmd
    res = run_bass_kernel_spmd(nc, in_maps, list(range(NCORES)))
    LAST_RESULTS = res
    return _unpack_output(res.results, counts, cfg, starts)
